# revision 30
# baseline (speedup 1.0000x reference)
"""MoE feed-forward (top-2 routing + shared expert) on 8 Trainium2 cores.

Strategy (expert parallel):
  - Host computes the router (tiny [T,D]@[D,E] matmul), top-2 expert ids and
    renormalized gates, then dispatches each expert's tokens (transposed,
    capacity-padded) to the core that owns that expert's weights.
  - Core e computes  ye = (silu(xe@w1_e) * (xe@w3_e)) @ w2_e, row-scaled by the
    gate, plus a 1/8 token-slice of the always-active shared expert.
  - Host scatter-adds routed outputs into the shared-expert output.

All matmuls run in bf16 (fp32 PSUM accumulation). bf16 keeps the PE at one
moving column per cycle like fp32r, but its LDWEIGHTS goes through the fast
weight load path (~53ns vs ~191ns), so narrow token chunks no longer pay a
weight-load floor, and every DMA byte count halves.

Dataflow per core: x (all C routed + S shared tokens, transposed) and the
swiglu gate buffer g live in SBUF for the whole kernel. Phase 1 runs h-tile
OUTER / token-chunk INNER so each w1/w3/sw1/sw3 tile streams from HBM exactly
once. Phase 2 (down-projection) runs against SBUF-resident w2/sw2.
"""

import numpy as np
import ml_dtypes

import concourse.bass as bass
import concourse.mybir as mybir
import concourse.tile as tile
from concourse import bacc
from concourse.bass_utils import run_bass_kernel_spmd

P = 128
N_CORES = 8
F32 = mybir.dt.float32
BF16 = mybir.dt.bfloat16
AF = mybir.ActivationFunctionType
BF16_NP = ml_dtypes.bfloat16

# h-tiles of w1/w3 fetched per DMA (bigger transfers, fewer descriptors)
H_BLOCK = 1


def _chunks(n):
    """Split n tokens into moving-operand chunks of <=512 columns."""
    out = []
    c0 = 0
    while c0 < n:
        cw = min(512, n - c0)
        out.append((c0, cw))
        c0 += cw
    return out


def _chunk_layout(C, S):
    """Chunk processing/packing order: (token offset, width, routed?).
    Narrowest routed chunks first so the startup matmuls wait on the
    least DMA data; shared-expert chunks last."""
    rc = _chunks(C)
    if len(rc) >= 2 and rc[-2][1] == 512:
        c0 = rc[-2][0]
        rc = rc[:-2] + [(c0, 256), (c0 + 256, 256), rc[-1]]
    return sorted([(c0, cw, True) for c0, cw in rc], key=lambda t: t[1]) + [
        (C + c0, cw, False) for c0, cw in _chunks(S)
    ]


def _ttiles(n):
    """Split n rows into output partition tiles of <=128."""
    out = []
    t0 = 0
    while t0 < n:
        tw = min(P, n - t0)
        out.append((t0, tw))
        t0 += tw
    return out


def build_moe_program(D, H, C, S, use_silu=True):
    """SPMD program: routed expert over C capacity rows + shared expert over
    S token-slice rows. Same NEFF on all 8 cores, per-core input data."""
    nc = bacc.Bacc(
        "TRN2", target_bir_lowering=False, debug=False, num_devices=N_CORES
    )

    KD = D // P
    KH = H // P
    ND = D // 512
    CT = C + S
    CP = (C + P - 1) // P * P  # gate tensor rows (128-multiple)
    hbsz = KD * 2 * H_BLOCK * P  # packed cols per (w1|w3) h-block pair

    def din(name, shape, dt=BF16):
        return nc.dram_tensor(name, shape, dt, kind="ExternalInput").ap()

    def dout(name, shape):
        return nc.dram_tensor(name, shape, BF16, kind="ExternalOutput").ap()

    xT = din("xT", [P, KD * CT])  # chunk-contiguous packed tokens
    ge = din("ge", [P, CP // P], F32)  # gates, pre-transposed on host
    w13 = din("w13", [P, KD * 2 * H])  # w1|w3 h-tile pairs
    w2 = din("w2", [P, KH * D])
    sw13 = din("sw13", [P, KD * 2 * H])
    sw2 = din("sw2", [P, KH * D])
    ye = dout("ye", [C, D])
    se = dout("se", [S, D])

    chunk_list = _chunk_layout(C, S)

    def _wsrc(ap, hb):
        return ap[:, hb * hbsz : (hb + 1) * hbsz].rearrange(
            "p (k m) -> p k m", k=KD
        )

    with tile.TileContext(nc) as tc:
        from contextlib import ExitStack

        with ExitStack() as ctx:
            xpool = ctx.enter_context(tc.tile_pool(name="xT", bufs=1))
            gpool = ctx.enter_context(tc.tile_pool(name="gbuf", bufs=1))
            w2pool = ctx.enter_context(tc.tile_pool(name="w2res", bufs=1))
            wpool = ctx.enter_context(tc.tile_pool(name="wstream", bufs=3))
            spool = ctx.enter_context(tc.tile_pool(name="stemp", bufs=2))
            opool = ctx.enter_context(tc.tile_pool(name="otile", bufs=3))
            gepool = ctx.enter_context(tc.tile_pool(name="gate", bufs=1))
            pp1 = ctx.enter_context(tc.tile_pool(name="ps1", bufs=3, space="PSUM"))
            pp3 = ctx.enter_context(tc.tile_pool(name="ps3", bufs=2, space="PSUM"))
            ppo = ctx.enter_context(tc.tile_pool(name="pso", bufs=2, space="PSUM"))

            # PE warmup: junk matmuls from t~0 so the HAM clock gate is
            # already at 2.4 GHz when the first real matmul issues (the PE
            # would otherwise run its first ~3.4us of work at 1.2 GHz, and
            # an idle PE while DMA streams would re-throttle it).
            jpool = ctx.enter_context(tc.tile_pool(name="junk", bufs=1))
            pjw = ctx.enter_context(tc.tile_pool(name="psw", bufs=1, space="PSUM"))
            jt = jpool.tile([P, 64], BF16, tag="jt", name="jt")
            nc.vector.memset(jt[:], 0)
            jp = pjw.tile([64, 64], F32, tag="jp", name="jp")
            for _ in range(210):
                nc.tensor.matmul(jp, jt[:, :64], jt[:, :64], start=True, stop=True)

            def _load_w13(srcp, hb, t1):
                wt = wpool.tile([P, KD, 2 * H_BLOCK * P], BF16, tag=t1, name="wt")
                nc.sync.dma_start(wt[:], _wsrc(srcp, hb))
                return wt

            # resident activations, chunk-contiguous per partition so every
            # chunk load is one contiguous run per partition. Trigger order:
            # hb0 routed weights, the two leading chunks, hb0 shared
            # weights, remaining chunks.
            xt = xpool.tile([P, KD * CT], BF16, tag="xt", name="xt")
            xoff = {}
            off = 0
            for c0, cw, _ in chunk_list:
                xoff[c0] = off
                off += KD * cw

            def _load_chunk(idx, eng):
                c0, cw, _ = chunk_list[idx]
                o = xoff[c0]
                if idx == 0:
                    # two k-halves: the first accumulation chain's k=0..3
                    # matmuls can start as soon as half the chunk landed
                    h0 = KD // 2 * cw
                    eng.dma_start(xt[:, o : o + h0], xT[:, o : o + h0])
                    eng.dma_start(
                        xt[:, o + h0 : o + KD * cw], xT[:, o + h0 : o + KD * cw]
                    )
                else:
                    eng.dma_start(xt[:, o : o + KD * cw], xT[:, o : o + KD * cw])

            def _xop(c0, cw):
                o = xoff[c0]
                return xt[:, o : o + KD * cw].rearrange("p (k c) -> p k c", k=KD)

            # Routed loads on the Sync HWDGE ring in consumption order
            # (the descriptor-generation rate ~215GB/s is the startup
            # bottleneck); the late-consumed shared x chunk and gates go on
            # the slow-but-parallel GpSimd SWDGE ring.
            tiles0 = _load_w13(w13, 0, "w13t")
            n_routed_chunks = sum(1 for c in chunk_list if c[2])
            for idx in range(n_routed_chunks):
                _load_chunk(idx, nc.sync)
            for idx in range(n_routed_chunks, len(chunk_list)):
                _load_chunk(idx, nc.gpsimd)
            stiles0 = _load_w13(sw13, 0, "sw13t")

            # per-token gates, [P, CP//P] pre-transposed on host
            get = gepool.tile([P, CP // P], F32, tag="ge", name="get")
            nc.gpsimd.dma_start(get[:], ge[:, :])

            # resident swiglu-gate buffer: [P, KH, CT] bf16
            gt = gpool.tile([P, KH, CT], BF16, tag="gt", name="gt")

            # resident down-projection weights (kh-major pack), loaded in
            # kh-quarter DMAs interleaved between hb iterations so they
            # never monopolize the descriptor ring ahead of stream tiles
            w2res = w2pool.tile([P, KH, D], BF16, tag="w2res", name="w2t")
            sw2res = w2pool.tile([P, KH, D], BF16, tag="sw2res", name="sw2t")
            KQ = KH // 4
            w2loads = [(w2res, w2, q) for q in range(4)] + [
                (sw2res, sw2, q) for q in range(4)
            ]

            # ---- phase 1: gt[h, t] = silu(h1T) * h3T, h-block outer ----
            for hb in range(KH // H_BLOCK):
                if hb == 0:
                    wt, swt = tiles0, stiles0
                else:
                    wt = _load_w13(w13, hb, "w13t")
                    swt = _load_w13(sw13, hb, "sw13t")

                if 4 <= hb < 4 + len(w2loads):
                    res, src, q = w2loads[hb - 4]
                    nc.sync.dma_start(
                        res[:, q * KQ : (q + 1) * KQ, :],
                        src[:, q * KQ * D : (q + 1) * KQ * D].rearrange(
                            "p (k m) -> p k m", k=KQ
                        ),
                    )

                for c0, cw, routed in chunk_list:
                    xop = _xop(c0, cw)
                    for hi in range(H_BLOCK):
                        h = hb * H_BLOCK + hi
                        wab = wt if routed else swt
                        p1 = pp1.tile([P, 512], F32, tag="p1", name="p1")[:, :cw]
                        for k in range(KD):
                            nc.tensor.matmul(
                                p1,
                                wab[:, k, 2 * hi * P : (2 * hi + 1) * P],
                                xop[:, k, :],
                                start=(k == 0),
                                stop=(k == KD - 1),
                            )
                        p3 = pp3.tile([P, 512], F32, tag="p3", name="p3")[:, :cw]
                        for k in range(KD):
                            nc.tensor.matmul(
                                p3,
                                wab[:, k, (2 * hi + 1) * P : (2 * hi + 2) * P],
                                xop[:, k, :],
                                start=(k == 0),
                                stop=(k == KD - 1),
                            )
                        gs = gt[:, h, c0 : c0 + cw]
                        if use_silu:
                            nc.scalar.activation(gs, p1, AF.Silu)
                            nc.vector.tensor_mul(gs, gs, p3)
                        else:  # silu(a) = a * sigmoid(a); CoreSim has no Silu
                            s1 = spool.tile([P, 512], F32, tag="s1", name="s1")[:, :cw]
                            nc.scalar.activation(s1, p1, AF.Sigmoid)
                            nc.vector.tensor_mul(s1, s1, p1)
                            nc.vector.tensor_mul(gs, s1, p3)

            # ---- phase 2: ye/se = gT.T @ w2, row-scaled by gate ----
            # routed first: the last output DMA is then a full-128-partition
            # tile, which sprays across all 16 SDMA engines (a partial-
            # partition tile as the final DMA drains through one engine and
            # adds ~10us of tail)
            for sec_routed in (True, False):
                n_rows = C if sec_routed else S
                base = 0 if sec_routed else C
                wres = w2res if sec_routed else sw2res
                out_ap = ye if sec_routed else se
                for t0, tw in _ttiles(n_rows):
                    for dn in range(ND):
                        po = ppo.tile([P, 512], F32, tag="po", name="po")[:tw, :]
                        for kh in range(KH):
                            nc.tensor.matmul(
                                po,
                                gt[:, kh, base + t0 : base + t0 + tw],
                                wres[:, kh, dn * 512 : (dn + 1) * 512],
                                start=(kh == 0),
                                stop=(kh == KH - 1),
                            )
                        ot = opool.tile([P, 512], BF16, tag="ot", name="ot")[:tw, :]
                        if sec_routed:
                            nc.vector.tensor_scalar_mul(
                                ot, po, get[:tw, t0 // P : t0 // P + 1]
                            )
                        else:
                            nc.vector.tensor_copy(ot, po)
                        nc.sync.dma_start(
                            out_ap[t0 : t0 + tw, dn * 512 : (dn + 1) * 512], ot
                        )

    nc.compile()
    return nc


_PROGRAM_CACHE = {}
LAST_RESULTS = None  # BassKernelResults of the most recent device run (for test.py)


def _get_program(D, H, C, S):
    key = (D, H, C, S)
    if key not in _PROGRAM_CACHE:
        _PROGRAM_CACHE[key] = build_moe_program(D, H, C, S)
    return _PROGRAM_CACHE[key]


def _pack_xT(xmat, C, S):
    """[C+S, D] row-major tokens -> [P, (D//P)*(C+S)] partition-major bf16,
    chunk-contiguous in _chunk_layout order."""
    n, D = xmat.shape
    KD = D // P
    xr = xmat.reshape(n, KD, P).transpose(2, 1, 0)  # [P, KD, n]
    out = np.empty((P, KD * n), BF16_NP)
    off = 0
    for c0, cw, _ in _chunk_layout(C, S):
        out[:, off : off + KD * cw] = xr[:, :, c0 : c0 + cw].reshape(P, KD * cw)
        off += KD * cw
    return out


def _pack_w13(wa, wb):
    """Two [D, H] weights -> [P, (D//P)*2H] bf16, h-tile-major with the
    w1|w3 tiles of each h paired so one DMA fetches both."""
    Dw, Hw = wa.shape
    KD = Dw // P
    KH = Hw // P
    s = np.stack(
        [wa.reshape(KD, P, KH, P), wb.reshape(KD, P, KH, P)], axis=3
    )  # [KD, P, KH, 2, P]
    return np.ascontiguousarray(
        s.transpose(1, 2, 0, 3, 4).reshape(P, KD * 2 * Hw)
    ).astype(BF16_NP)


def _pack_w2(w):
    """[H, D] -> [P, H*D//P] kh-major bf16: one contiguous run per
    partition, loadable in a single DMA."""
    Hw, Dw = w.shape
    KH = Hw // P
    return np.ascontiguousarray(
        w.reshape(KH, P, Dw).transpose(1, 0, 2).reshape(P, Hw * Dw // P)
    ).astype(BF16_NP)


def _route(xf, w_router):
    """Top-2 routing identical (up to fp rounding) to the jax reference."""
    logits = xf @ w_router.astype(np.float32)  # [T, E]
    # softmax is monotone: top-2 of probs == top-2 of logits, stable ties
    top2 = np.argsort(-logits, axis=1, kind="stable")[:, :2]  # [T, 2]
    lv = np.take_along_axis(logits, top2, axis=1)
    ev = np.exp(lv - lv[:, 0:1])
    gates = ev / ev.sum(axis=1, keepdims=True)  # [T, 2] renormalized
    return top2, gates


def kernel(x, w_router, w1, w3, w2, sw1, sw3, sw2):
    B, SEQ, D = x.shape
    T = B * SEQ
    E, _, H = w1.shape
    assert E == N_CORES
    S = T // N_CORES

    x = np.asarray(x, dtype=np.float32)
    xf = np.ascontiguousarray(x.reshape(T, D))
    top2, gates = _route(xf, np.asarray(w_router, np.float32))

    # per-expert token lists + gate values
    flat_e = top2.ravel()  # slot 2t, 2t+1 -> token t
    flat_g = gates.ravel().astype(np.float32)
    order = np.argsort(flat_e, kind="stable")
    sorted_e = flat_e[order]
    starts = np.searchsorted(sorted_e, np.arange(E + 1))
    tok_by_e = [order[starts[e] : starts[e + 1]] >> 1 for e in range(E)]
    gate_by_e = [flat_g[order[starts[e] : starts[e + 1]]] for e in range(E)]
    counts = np.diff(starts)

    # Device capacity = the mean expert load (zero padding waste); the few
    # over-capacity slots of hot experts (~1.5% here) are handled on the
    # host exactly, like a fixed-capacity MoE dispatch overflow path.
    cap = T * 2 // E
    C = max(512, min(cap, (int(counts.max()) + P - 1) // P * P))
    CP = (C + P - 1) // P * P

    nc = _get_program(D, H, C, S)

    w1 = np.asarray(w1, np.float32)
    w3 = np.asarray(w3, np.float32)
    w2 = np.asarray(w2, np.float32)
    sw13p = _pack_w13(np.asarray(sw1, np.float32), np.asarray(sw3, np.float32))
    sw2p = _pack_w2(np.asarray(sw2, np.float32))

    in_maps = []
    for e in range(E):
        n_e = min(int(counts[e]), C)
        xe_pad = np.zeros((C + S, D), np.float32)
        xe_pad[:n_e] = xf[tok_by_e[e][:n_e]]
        xe_pad[C:] = xf[e * S : (e + 1) * S]
        ge = np.zeros(CP, np.float32)
        ge[:n_e] = gate_by_e[e][:n_e]
        in_maps.append(
            {
                "xT": _pack_xT(xe_pad, C, S),
                "ge": np.ascontiguousarray(ge.reshape(CP // P, P).T),
                "w13": _pack_w13(w1[e], w3[e]),
                "w2": _pack_w2(w2[e]),
                "sw13": sw13p,
                "sw2": sw2p,
            }
        )

    global LAST_RESULTS
    LAST_RESULTS = run_bass_kernel_spmd(nc, in_maps, core_ids=list(range(N_CORES)))
    res = LAST_RESULTS.results

    out = np.empty((T, D), np.float32)
    for c in range(N_CORES):
        out[c * S : (c + 1) * S] = np.asarray(res[c]["se"], np.float32)
    for e in range(E):
        n_e = min(int(counts[e]), C)
        if n_e:
            out[tok_by_e[e][:n_e]] += np.asarray(res[e]["ye"][:n_e], np.float32)
        if int(counts[e]) > C:  # capacity-overflow slots, computed exactly
            idx = tok_by_e[e][C:]
            g = gate_by_e[e][C:]
            xo = xf[idx]
            h1 = xo @ w1[e]
            yo = (h1 / (1.0 + np.exp(-h1)) * (xo @ w3[e])) @ w2[e]
            out[idx] += yo * g[:, None]
    return out.reshape(B, SEQ, D)


# revision 31
# speedup vs baseline: 1.0044x; 1.0044x over previous
"""MoE feed-forward (top-2 routing + shared expert) on 8 Trainium2 cores.

Strategy (expert parallel):
  - Host computes the router (tiny [T,D]@[D,E] matmul), top-2 expert ids and
    renormalized gates, then dispatches each expert's tokens (transposed,
    capacity-padded) to the core that owns that expert's weights.
  - Core e computes  ye = (silu(xe@w1_e) * (xe@w3_e)) @ w2_e, row-scaled by the
    gate, plus a 1/8 token-slice of the always-active shared expert.
  - Host scatter-adds routed outputs into the shared-expert output.

All matmuls run in bf16 (fp32 PSUM accumulation). bf16 keeps the PE at one
moving column per cycle like fp32r, but its LDWEIGHTS goes through the fast
weight load path (~53ns vs ~191ns), so narrow token chunks no longer pay a
weight-load floor, and every DMA byte count halves.

Dataflow per core: x (all C routed + S shared tokens, transposed) and the
swiglu gate buffer g live in SBUF for the whole kernel. Phase 1 runs h-tile
OUTER / token-chunk INNER so each w1/w3/sw1/sw3 tile streams from HBM exactly
once. Phase 2 (down-projection) runs against SBUF-resident w2/sw2.
"""

import numpy as np
import ml_dtypes

import concourse.bass as bass
import concourse.mybir as mybir
import concourse.tile as tile
from concourse import bacc
from concourse.bass_utils import run_bass_kernel_spmd

P = 128
N_CORES = 8
F32 = mybir.dt.float32
BF16 = mybir.dt.bfloat16
AF = mybir.ActivationFunctionType
BF16_NP = ml_dtypes.bfloat16

# h-tiles of w1/w3 fetched per DMA (bigger transfers, fewer descriptors)
H_BLOCK = 1


def _chunks(n):
    """Split n tokens into moving-operand chunks of <=512 columns."""
    out = []
    c0 = 0
    while c0 < n:
        cw = min(512, n - c0)
        out.append((c0, cw))
        c0 += cw
    return out


def _chunk_layout(C, S):
    """Chunk processing/packing order: (token offset, width, routed?).
    Narrowest routed chunks first so the startup matmuls wait on the
    least DMA data; shared-expert chunks last."""
    rc = _chunks(C)
    if len(rc) >= 2 and rc[-2][1] == 512:
        c0 = rc[-2][0]
        rc = rc[:-2] + [(c0, 256), (c0 + 256, 256), rc[-1]]
    return sorted([(c0, cw, True) for c0, cw in rc], key=lambda t: t[1]) + [
        (C + c0, cw, False) for c0, cw in _chunks(S)
    ]


def _ttiles(n):
    """Split n rows into output partition tiles of <=128."""
    out = []
    t0 = 0
    while t0 < n:
        tw = min(P, n - t0)
        out.append((t0, tw))
        t0 += tw
    return out


def build_moe_program(D, H, C, S, use_silu=True):
    """SPMD program: routed expert over C capacity rows + shared expert over
    S token-slice rows. Same NEFF on all 8 cores, per-core input data."""
    nc = bacc.Bacc(
        "TRN2", target_bir_lowering=False, debug=False, num_devices=N_CORES
    )

    KD = D // P
    KH = H // P
    ND = D // 512
    CT = C + S
    CP = (C + P - 1) // P * P  # gate tensor rows (128-multiple)
    hbsz = KD * 2 * H_BLOCK * P  # packed cols per (w1|w3) h-block pair

    def din(name, shape, dt=BF16):
        return nc.dram_tensor(name, shape, dt, kind="ExternalInput").ap()

    def dout(name, shape):
        return nc.dram_tensor(name, shape, BF16, kind="ExternalOutput").ap()

    xT = din("xT", [P, KD * CT])  # chunk-contiguous packed tokens
    ge = din("ge", [P, CP // P], F32)  # gates, pre-transposed on host
    w13 = din("w13", [P, KD * 2 * H])  # w1|w3 h-tile pairs
    w2 = din("w2", [P, KH * D])
    sw13 = din("sw13", [P, KD * 2 * H])
    sw2 = din("sw2", [P, KH * D])
    ye = dout("ye", [C, D])
    se = dout("se", [S, D])

    chunk_list = _chunk_layout(C, S)

    def _wsrc(ap, hb):
        return ap[:, hb * hbsz : (hb + 1) * hbsz].rearrange(
            "p (k m) -> p k m", k=KD
        )

    with tile.TileContext(nc) as tc:
        from contextlib import ExitStack

        with ExitStack() as ctx:
            xpool = ctx.enter_context(tc.tile_pool(name="xT", bufs=1))
            gpool = ctx.enter_context(tc.tile_pool(name="gbuf", bufs=1))
            w2pool = ctx.enter_context(tc.tile_pool(name="w2res", bufs=1))
            wpool = ctx.enter_context(tc.tile_pool(name="wstream", bufs=3))
            spool = ctx.enter_context(tc.tile_pool(name="stemp", bufs=2))
            opool = ctx.enter_context(tc.tile_pool(name="otile", bufs=3))
            gepool = ctx.enter_context(tc.tile_pool(name="gate", bufs=1))
            pp1 = ctx.enter_context(tc.tile_pool(name="ps1", bufs=3, space="PSUM"))
            pp3 = ctx.enter_context(tc.tile_pool(name="ps3", bufs=2, space="PSUM"))
            ppo = ctx.enter_context(tc.tile_pool(name="pso", bufs=2, space="PSUM"))

            # PE warmup: junk matmuls from t~0 so the HAM clock gate is
            # already at 2.4 GHz when the first real matmul issues (the PE
            # would otherwise run its first ~3.4us of work at 1.2 GHz, and
            # an idle PE while DMA streams would re-throttle it).
            jpool = ctx.enter_context(tc.tile_pool(name="junk", bufs=1))
            pjw = ctx.enter_context(tc.tile_pool(name="psw", bufs=1, space="PSUM"))
            jt = jpool.tile([P, 64], BF16, tag="jt", name="jt")
            nc.vector.memset(jt[:], 0)
            jp = pjw.tile([64, 64], F32, tag="jp", name="jp")
            for _ in range(210):
                nc.tensor.matmul(jp, jt[:, :64], jt[:, :64], start=True, stop=True)

            def _load_w13(srcp, hb, t1):
                wt = wpool.tile([P, KD, 2 * H_BLOCK * P], BF16, tag=t1, name="wt")
                nc.sync.dma_start(wt[:], _wsrc(srcp, hb))
                return wt

            # resident activations, chunk-contiguous per partition so every
            # chunk load is one contiguous run per partition. Trigger order:
            # hb0 routed weights, the two leading chunks, hb0 shared
            # weights, remaining chunks.
            xt = xpool.tile([P, KD * CT], BF16, tag="xt", name="xt")
            xoff = {}
            off = 0
            for c0, cw, _ in chunk_list:
                xoff[c0] = off
                off += KD * cw

            def _load_chunk(idx, eng):
                c0, cw, _ = chunk_list[idx]
                o = xoff[c0]
                if idx == 0:
                    # two k-halves: the first accumulation chain's k=0..3
                    # matmuls can start as soon as half the chunk landed
                    h0 = KD // 2 * cw
                    eng.dma_start(xt[:, o : o + h0], xT[:, o : o + h0])
                    eng.dma_start(
                        xt[:, o + h0 : o + KD * cw], xT[:, o + h0 : o + KD * cw]
                    )
                else:
                    eng.dma_start(xt[:, o : o + KD * cw], xT[:, o : o + KD * cw])

            def _xop(c0, cw):
                o = xoff[c0]
                return xt[:, o : o + KD * cw].rearrange("p (k c) -> p k c", k=KD)

            # Routed loads on the Sync HWDGE ring in consumption order
            # (the descriptor-generation rate ~215GB/s is the startup
            # bottleneck); the late-consumed shared x chunk and gates go on
            # the slow-but-parallel GpSimd SWDGE ring.
            tiles0 = _load_w13(w13, 0, "w13t")
            n_routed_chunks = sum(1 for c in chunk_list if c[2])
            for idx in range(n_routed_chunks):
                _load_chunk(idx, nc.sync)
            for idx in range(n_routed_chunks, len(chunk_list)):
                _load_chunk(idx, nc.gpsimd)
            stiles0 = _load_w13(sw13, 0, "sw13t")

            # per-token gates, [P, CP//P] pre-transposed on host
            get = gepool.tile([P, CP // P], F32, tag="ge", name="get")
            nc.gpsimd.dma_start(get[:], ge[:, :])

            # resident swiglu-gate buffer: [P, KH, CT] bf16
            gt = gpool.tile([P, KH, CT], BF16, tag="gt", name="gt")

            # resident down-projection weights (kh-major pack), loaded in
            # kh-quarter DMAs interleaved between hb iterations so they
            # never monopolize the descriptor ring ahead of stream tiles
            w2res = w2pool.tile([P, KH, D], BF16, tag="w2res", name="w2t")
            sw2res = w2pool.tile([P, KH, D], BF16, tag="sw2res", name="sw2t")
            KQ = KH // 4
            w2loads = [(w2res, w2, q) for q in range(4)] + [
                (sw2res, sw2, q) for q in range(4)
            ]

            # ---- phase 1: gt[h, t] = silu(h1T) * h3T, h-block outer ----
            for hb in range(KH // H_BLOCK):
                if hb == 0:
                    wt, swt = tiles0, stiles0
                else:
                    wt = _load_w13(w13, hb, "w13t")
                    swt = _load_w13(sw13, hb, "sw13t")

                if 4 <= hb < 4 + len(w2loads):
                    res, src, q = w2loads[hb - 4]
                    nc.sync.dma_start(
                        res[:, q * KQ : (q + 1) * KQ, :],
                        src[:, q * KQ * D : (q + 1) * KQ * D].rearrange(
                            "p (k m) -> p k m", k=KQ
                        ),
                    )

                for c0, cw, routed in chunk_list:
                    xop = _xop(c0, cw)
                    for hi in range(H_BLOCK):
                        h = hb * H_BLOCK + hi
                        wab = wt if routed else swt
                        p1 = pp1.tile([P, 512], F32, tag="p1", name="p1")[:, :cw]
                        for k in range(KD):
                            nc.tensor.matmul(
                                p1,
                                wab[:, k, 2 * hi * P : (2 * hi + 1) * P],
                                xop[:, k, :],
                                start=(k == 0),
                                stop=(k == KD - 1),
                            )
                        p3 = pp3.tile([P, 512], F32, tag="p3", name="p3")[:, :cw]
                        for k in range(KD):
                            nc.tensor.matmul(
                                p3,
                                wab[:, k, (2 * hi + 1) * P : (2 * hi + 2) * P],
                                xop[:, k, :],
                                start=(k == 0),
                                stop=(k == KD - 1),
                            )
                        gs = gt[:, h, c0 : c0 + cw]
                        if use_silu:
                            s1 = spool.tile([P, 512], F32, tag="s1", name="s1")[:, :cw]
                            nc.scalar.activation(s1, p1, AF.Silu)
                            nc.vector.tensor_mul(gs, s1, p3)
                        else:  # silu(a) = a * sigmoid(a); CoreSim has no Silu
                            s1 = spool.tile([P, 512], F32, tag="s1", name="s1")[:, :cw]
                            nc.scalar.activation(s1, p1, AF.Sigmoid)
                            nc.vector.tensor_mul(s1, s1, p1)
                            nc.vector.tensor_mul(gs, s1, p3)

            # ---- phase 2: ye/se = gT.T @ w2, row-scaled by gate ----
            # routed first: the last output DMA is then a full-128-partition
            # tile, which sprays across all 16 SDMA engines (a partial-
            # partition tile as the final DMA drains through one engine and
            # adds ~10us of tail)
            for sec_routed in (True, False):
                n_rows = C if sec_routed else S
                base = 0 if sec_routed else C
                wres = w2res if sec_routed else sw2res
                out_ap = ye if sec_routed else se
                for t0, tw in _ttiles(n_rows):
                    for dn in range(ND):
                        po = ppo.tile([P, 512], F32, tag="po", name="po")[:tw, :]
                        for kh in range(KH):
                            nc.tensor.matmul(
                                po,
                                gt[:, kh, base + t0 : base + t0 + tw],
                                wres[:, kh, dn * 512 : (dn + 1) * 512],
                                start=(kh == 0),
                                stop=(kh == KH - 1),
                            )
                        ot = opool.tile([P, 512], BF16, tag="ot", name="ot")[:tw, :]
                        if sec_routed:
                            nc.vector.tensor_scalar_mul(
                                ot, po, get[:tw, t0 // P : t0 // P + 1]
                            )
                        else:
                            nc.vector.tensor_copy(ot, po)
                        nc.sync.dma_start(
                            out_ap[t0 : t0 + tw, dn * 512 : (dn + 1) * 512], ot
                        )

    nc.compile()
    return nc


_PROGRAM_CACHE = {}
LAST_RESULTS = None  # BassKernelResults of the most recent device run (for test.py)


def _get_program(D, H, C, S):
    key = (D, H, C, S)
    if key not in _PROGRAM_CACHE:
        _PROGRAM_CACHE[key] = build_moe_program(D, H, C, S)
    return _PROGRAM_CACHE[key]


def _pack_xT(xmat, C, S):
    """[C+S, D] row-major tokens -> [P, (D//P)*(C+S)] partition-major bf16,
    chunk-contiguous in _chunk_layout order."""
    n, D = xmat.shape
    KD = D // P
    xr = xmat.reshape(n, KD, P).transpose(2, 1, 0)  # [P, KD, n]
    out = np.empty((P, KD * n), BF16_NP)
    off = 0
    for c0, cw, _ in _chunk_layout(C, S):
        out[:, off : off + KD * cw] = xr[:, :, c0 : c0 + cw].reshape(P, KD * cw)
        off += KD * cw
    return out


def _pack_w13(wa, wb):
    """Two [D, H] weights -> [P, (D//P)*2H] bf16, h-tile-major with the
    w1|w3 tiles of each h paired so one DMA fetches both."""
    Dw, Hw = wa.shape
    KD = Dw // P
    KH = Hw // P
    s = np.stack(
        [wa.reshape(KD, P, KH, P), wb.reshape(KD, P, KH, P)], axis=3
    )  # [KD, P, KH, 2, P]
    return np.ascontiguousarray(
        s.transpose(1, 2, 0, 3, 4).reshape(P, KD * 2 * Hw)
    ).astype(BF16_NP)


def _pack_w2(w):
    """[H, D] -> [P, H*D//P] kh-major bf16: one contiguous run per
    partition, loadable in a single DMA."""
    Hw, Dw = w.shape
    KH = Hw // P
    return np.ascontiguousarray(
        w.reshape(KH, P, Dw).transpose(1, 0, 2).reshape(P, Hw * Dw // P)
    ).astype(BF16_NP)


def _route(xf, w_router):
    """Top-2 routing identical (up to fp rounding) to the jax reference."""
    logits = xf @ w_router.astype(np.float32)  # [T, E]
    # softmax is monotone: top-2 of probs == top-2 of logits, stable ties
    top2 = np.argsort(-logits, axis=1, kind="stable")[:, :2]  # [T, 2]
    lv = np.take_along_axis(logits, top2, axis=1)
    ev = np.exp(lv - lv[:, 0:1])
    gates = ev / ev.sum(axis=1, keepdims=True)  # [T, 2] renormalized
    return top2, gates


def kernel(x, w_router, w1, w3, w2, sw1, sw3, sw2):
    B, SEQ, D = x.shape
    T = B * SEQ
    E, _, H = w1.shape
    assert E == N_CORES
    S = T // N_CORES

    x = np.asarray(x, dtype=np.float32)
    xf = np.ascontiguousarray(x.reshape(T, D))
    top2, gates = _route(xf, np.asarray(w_router, np.float32))

    # per-expert token lists + gate values
    flat_e = top2.ravel()  # slot 2t, 2t+1 -> token t
    flat_g = gates.ravel().astype(np.float32)
    order = np.argsort(flat_e, kind="stable")
    sorted_e = flat_e[order]
    starts = np.searchsorted(sorted_e, np.arange(E + 1))
    tok_by_e = [order[starts[e] : starts[e + 1]] >> 1 for e in range(E)]
    gate_by_e = [flat_g[order[starts[e] : starts[e + 1]]] for e in range(E)]
    counts = np.diff(starts)

    # Device capacity = the mean expert load (zero padding waste); the few
    # over-capacity slots of hot experts (~1.5% here) are handled on the
    # host exactly, like a fixed-capacity MoE dispatch overflow path.
    cap = T * 2 // E
    C = max(512, min(cap, (int(counts.max()) + P - 1) // P * P))
    CP = (C + P - 1) // P * P

    nc = _get_program(D, H, C, S)

    w1 = np.asarray(w1, np.float32)
    w3 = np.asarray(w3, np.float32)
    w2 = np.asarray(w2, np.float32)
    sw13p = _pack_w13(np.asarray(sw1, np.float32), np.asarray(sw3, np.float32))
    sw2p = _pack_w2(np.asarray(sw2, np.float32))

    in_maps = []
    for e in range(E):
        n_e = min(int(counts[e]), C)
        xe_pad = np.zeros((C + S, D), np.float32)
        xe_pad[:n_e] = xf[tok_by_e[e][:n_e]]
        xe_pad[C:] = xf[e * S : (e + 1) * S]
        ge = np.zeros(CP, np.float32)
        ge[:n_e] = gate_by_e[e][:n_e]
        in_maps.append(
            {
                "xT": _pack_xT(xe_pad, C, S),
                "ge": np.ascontiguousarray(ge.reshape(CP // P, P).T),
                "w13": _pack_w13(w1[e], w3[e]),
                "w2": _pack_w2(w2[e]),
                "sw13": sw13p,
                "sw2": sw2p,
            }
        )

    global LAST_RESULTS
    LAST_RESULTS = run_bass_kernel_spmd(nc, in_maps, core_ids=list(range(N_CORES)))
    res = LAST_RESULTS.results

    out = np.empty((T, D), np.float32)
    for c in range(N_CORES):
        out[c * S : (c + 1) * S] = np.asarray(res[c]["se"], np.float32)
    for e in range(E):
        n_e = min(int(counts[e]), C)
        if n_e:
            out[tok_by_e[e][:n_e]] += np.asarray(res[e]["ye"][:n_e], np.float32)
        if int(counts[e]) > C:  # capacity-overflow slots, computed exactly
            idx = tok_by_e[e][C:]
            g = gate_by_e[e][C:]
            xo = xf[idx]
            h1 = xo @ w1[e]
            yo = (h1 / (1.0 + np.exp(-h1)) * (xo @ w3[e])) @ w2[e]
            out[idx] += yo * g[:, None]
    return out.reshape(B, SEQ, D)


# revision 32
# speedup vs baseline: 1.0070x; 1.0026x over previous
"""MoE feed-forward (top-2 routing + shared expert) on 8 Trainium2 cores.

Strategy (expert parallel):
  - Host computes the router (tiny [T,D]@[D,E] matmul), top-2 expert ids and
    renormalized gates, then dispatches each expert's tokens (transposed,
    capacity-padded) to the core that owns that expert's weights.
  - Core e computes  ye = (silu(xe@w1_e) * (xe@w3_e)) @ w2_e, row-scaled by the
    gate, plus a 1/8 token-slice of the always-active shared expert.
  - Host scatter-adds routed outputs into the shared-expert output.

All matmuls run in bf16 (fp32 PSUM accumulation). bf16 keeps the PE at one
moving column per cycle like fp32r, but its LDWEIGHTS goes through the fast
weight load path (~53ns vs ~191ns), so narrow token chunks no longer pay a
weight-load floor, and every DMA byte count halves.

Dataflow per core: x (all C routed + S shared tokens, transposed) and the
swiglu gate buffer g live in SBUF for the whole kernel. Phase 1 runs h-tile
OUTER / token-chunk INNER so each w1/w3/sw1/sw3 tile streams from HBM exactly
once. Phase 2 (down-projection) runs against SBUF-resident w2/sw2.
"""

import numpy as np
import ml_dtypes

import concourse.bass as bass
import concourse.mybir as mybir
import concourse.tile as tile
from concourse import bacc
from concourse.bass_utils import run_bass_kernel_spmd

P = 128
N_CORES = 8
F32 = mybir.dt.float32
BF16 = mybir.dt.bfloat16
AF = mybir.ActivationFunctionType
BF16_NP = ml_dtypes.bfloat16

# h-tiles of w1/w3 fetched per DMA (bigger transfers, fewer descriptors)
H_BLOCK = 1


def _chunks(n):
    """Split n tokens into moving-operand chunks of <=512 columns."""
    out = []
    c0 = 0
    while c0 < n:
        cw = min(512, n - c0)
        out.append((c0, cw))
        c0 += cw
    return out


def _chunk_layout(C, S):
    """Chunk processing/packing order: (token offset, width, routed?).
    Narrowest routed chunks first so the startup matmuls wait on the
    least DMA data; shared-expert chunks last."""
    rc = _chunks(C)
    return sorted([(c0, cw, True) for c0, cw in rc], key=lambda t: t[1]) + [
        (C + c0, cw, False) for c0, cw in _chunks(S)
    ]


def _ttiles(n):
    """Split n rows into output partition tiles of <=128."""
    out = []
    t0 = 0
    while t0 < n:
        tw = min(P, n - t0)
        out.append((t0, tw))
        t0 += tw
    return out


def build_moe_program(D, H, C, S, use_silu=True):
    """SPMD program: routed expert over C capacity rows + shared expert over
    S token-slice rows. Same NEFF on all 8 cores, per-core input data."""
    nc = bacc.Bacc(
        "TRN2", target_bir_lowering=False, debug=False, num_devices=N_CORES
    )

    KD = D // P
    KH = H // P
    ND = D // 512
    CT = C + S
    CP = (C + P - 1) // P * P  # gate tensor rows (128-multiple)
    hbsz = KD * 2 * H_BLOCK * P  # packed cols per (w1|w3) h-block pair

    def din(name, shape, dt=BF16):
        return nc.dram_tensor(name, shape, dt, kind="ExternalInput").ap()

    def dout(name, shape):
        return nc.dram_tensor(name, shape, BF16, kind="ExternalOutput").ap()

    xT = din("xT", [P, KD * CT])  # chunk-contiguous packed tokens
    ge = din("ge", [P, CP // P], F32)  # gates, pre-transposed on host
    w13 = din("w13", [P, KD * 2 * H])  # w1|w3 h-tile pairs
    w2 = din("w2", [P, KH * D])
    sw13 = din("sw13", [P, KD * 2 * H])
    sw2 = din("sw2", [P, KH * D])
    ye = dout("ye", [C, D])
    se = dout("se", [S, D])

    chunk_list = _chunk_layout(C, S)

    def _wsrc(ap, hb):
        return ap[:, hb * hbsz : (hb + 1) * hbsz].rearrange(
            "p (k m) -> p k m", k=KD
        )

    with tile.TileContext(nc) as tc:
        from contextlib import ExitStack

        with ExitStack() as ctx:
            xpool = ctx.enter_context(tc.tile_pool(name="xT", bufs=1))
            gpool = ctx.enter_context(tc.tile_pool(name="gbuf", bufs=1))
            w2pool = ctx.enter_context(tc.tile_pool(name="w2res", bufs=1))
            wpool = ctx.enter_context(tc.tile_pool(name="wstream", bufs=3))
            spool = ctx.enter_context(tc.tile_pool(name="stemp", bufs=2))
            opool = ctx.enter_context(tc.tile_pool(name="otile", bufs=3))
            gepool = ctx.enter_context(tc.tile_pool(name="gate", bufs=1))
            pp1 = ctx.enter_context(tc.tile_pool(name="ps1", bufs=3, space="PSUM"))
            pp3 = ctx.enter_context(tc.tile_pool(name="ps3", bufs=2, space="PSUM"))
            ppo = ctx.enter_context(tc.tile_pool(name="pso", bufs=2, space="PSUM"))

            # PE warmup: junk matmuls from t~0 so the HAM clock gate is
            # already at 2.4 GHz when the first real matmul issues (the PE
            # would otherwise run its first ~3.4us of work at 1.2 GHz, and
            # an idle PE while DMA streams would re-throttle it).
            jpool = ctx.enter_context(tc.tile_pool(name="junk", bufs=1))
            pjw = ctx.enter_context(tc.tile_pool(name="psw", bufs=1, space="PSUM"))
            jt = jpool.tile([P, 64], BF16, tag="jt", name="jt")
            nc.vector.memset(jt[:], 0)
            jp = pjw.tile([64, 64], F32, tag="jp", name="jp")
            for _ in range(170):
                nc.tensor.matmul(jp, jt[:, :64], jt[:, :64], start=True, stop=True)

            def _load_w13(srcp, hb, t1):
                wt = wpool.tile([P, KD, 2 * H_BLOCK * P], BF16, tag=t1, name="wt")
                nc.sync.dma_start(wt[:], _wsrc(srcp, hb))
                return wt

            # resident activations, chunk-contiguous per partition so every
            # chunk load is one contiguous run per partition. Trigger order:
            # hb0 routed weights, the two leading chunks, hb0 shared
            # weights, remaining chunks.
            xt = xpool.tile([P, KD * CT], BF16, tag="xt", name="xt")
            xoff = {}
            off = 0
            for c0, cw, _ in chunk_list:
                xoff[c0] = off
                off += KD * cw

            def _load_chunk(idx, eng):
                c0, cw, _ = chunk_list[idx]
                o = xoff[c0]
                if idx == 0:
                    # four k-quarters: the first accumulation chain's k=0..1
                    # matmuls can start as soon as a quarter chunk landed
                    q = KD // 4 * cw
                    for j in range(4):
                        eng.dma_start(
                            xt[:, o + j * q : o + (j + 1) * q],
                            xT[:, o + j * q : o + (j + 1) * q],
                        )
                else:
                    eng.dma_start(xt[:, o : o + KD * cw], xT[:, o : o + KD * cw])

            def _xop(c0, cw):
                o = xoff[c0]
                return xt[:, o : o + KD * cw].rearrange("p (k c) -> p k c", k=KD)

            # Routed loads on the Sync HWDGE ring in consumption order
            # (the descriptor-generation rate ~215GB/s is the startup
            # bottleneck); the late-consumed shared x chunk and gates go on
            # the slow-but-parallel GpSimd SWDGE ring.
            tiles0 = _load_w13(w13, 0, "w13t")
            n_routed_chunks = sum(1 for c in chunk_list if c[2])
            for idx in range(n_routed_chunks):
                _load_chunk(idx, nc.sync)
            for idx in range(n_routed_chunks, len(chunk_list)):
                _load_chunk(idx, nc.gpsimd)
            stiles0 = _load_w13(sw13, 0, "sw13t")

            # per-token gates, [P, CP//P] pre-transposed on host
            get = gepool.tile([P, CP // P], F32, tag="ge", name="get")
            nc.gpsimd.dma_start(get[:], ge[:, :])

            # resident swiglu-gate buffer: [P, KH, CT] bf16
            gt = gpool.tile([P, KH, CT], BF16, tag="gt", name="gt")

            # resident down-projection weights (kh-major pack), loaded in
            # kh-quarter DMAs interleaved between hb iterations so they
            # never monopolize the descriptor ring ahead of stream tiles
            w2res = w2pool.tile([P, KH, D], BF16, tag="w2res", name="w2t")
            sw2res = w2pool.tile([P, KH, D], BF16, tag="sw2res", name="sw2t")
            KQ = KH // 4
            w2loads = [(w2res, w2, q) for q in range(4)] + [
                (sw2res, sw2, q) for q in range(4)
            ]

            # ---- phase 1: gt[h, t] = silu(h1T) * h3T, h-block outer ----
            for hb in range(KH // H_BLOCK):
                if hb == 0:
                    wt, swt = tiles0, stiles0
                else:
                    wt = _load_w13(w13, hb, "w13t")
                    swt = _load_w13(sw13, hb, "sw13t")

                if 4 <= hb < 4 + len(w2loads):
                    res, src, q = w2loads[hb - 4]
                    nc.sync.dma_start(
                        res[:, q * KQ : (q + 1) * KQ, :],
                        src[:, q * KQ * D : (q + 1) * KQ * D].rearrange(
                            "p (k m) -> p k m", k=KQ
                        ),
                    )

                for c0, cw, routed in chunk_list:
                    xop = _xop(c0, cw)
                    for hi in range(H_BLOCK):
                        h = hb * H_BLOCK + hi
                        wab = wt if routed else swt
                        p1 = pp1.tile([P, 512], F32, tag="p1", name="p1")[:, :cw]
                        for k in range(KD):
                            nc.tensor.matmul(
                                p1,
                                wab[:, k, 2 * hi * P : (2 * hi + 1) * P],
                                xop[:, k, :],
                                start=(k == 0),
                                stop=(k == KD - 1),
                            )
                        p3 = pp3.tile([P, 512], F32, tag="p3", name="p3")[:, :cw]
                        for k in range(KD):
                            nc.tensor.matmul(
                                p3,
                                wab[:, k, (2 * hi + 1) * P : (2 * hi + 2) * P],
                                xop[:, k, :],
                                start=(k == 0),
                                stop=(k == KD - 1),
                            )
                        gs = gt[:, h, c0 : c0 + cw]
                        if use_silu:
                            s1 = spool.tile([P, 512], F32, tag="s1", name="s1")[:, :cw]
                            nc.scalar.activation(s1, p1, AF.Silu)
                            nc.vector.tensor_mul(gs, s1, p3)
                        else:  # silu(a) = a * sigmoid(a); CoreSim has no Silu
                            s1 = spool.tile([P, 512], F32, tag="s1", name="s1")[:, :cw]
                            nc.scalar.activation(s1, p1, AF.Sigmoid)
                            nc.vector.tensor_mul(s1, s1, p1)
                            nc.vector.tensor_mul(gs, s1, p3)

            # ---- phase 2: ye/se = gT.T @ w2, row-scaled by gate ----
            # routed first: the last output DMA is then a full-128-partition
            # tile, which sprays across all 16 SDMA engines (a partial-
            # partition tile as the final DMA drains through one engine and
            # adds ~10us of tail)
            for sec_routed in (True, False):
                n_rows = C if sec_routed else S
                base = 0 if sec_routed else C
                wres = w2res if sec_routed else sw2res
                out_ap = ye if sec_routed else se
                for t0, tw in _ttiles(n_rows):
                    for dn in range(ND):
                        po = ppo.tile([P, 512], F32, tag="po", name="po")[:tw, :]
                        for kh in range(KH):
                            nc.tensor.matmul(
                                po,
                                gt[:, kh, base + t0 : base + t0 + tw],
                                wres[:, kh, dn * 512 : (dn + 1) * 512],
                                start=(kh == 0),
                                stop=(kh == KH - 1),
                            )
                        ot = opool.tile([P, 512], BF16, tag="ot", name="ot")[:tw, :]
                        if sec_routed:
                            nc.vector.tensor_scalar_mul(
                                ot, po, get[:tw, t0 // P : t0 // P + 1]
                            )
                        else:
                            nc.vector.tensor_copy(ot, po)
                        nc.sync.dma_start(
                            out_ap[t0 : t0 + tw, dn * 512 : (dn + 1) * 512], ot
                        )

    nc.compile()
    return nc


_PROGRAM_CACHE = {}
LAST_RESULTS = None  # BassKernelResults of the most recent device run (for test.py)


def _get_program(D, H, C, S):
    key = (D, H, C, S)
    if key not in _PROGRAM_CACHE:
        _PROGRAM_CACHE[key] = build_moe_program(D, H, C, S)
    return _PROGRAM_CACHE[key]


def _pack_xT(xmat, C, S):
    """[C+S, D] row-major tokens -> [P, (D//P)*(C+S)] partition-major bf16,
    chunk-contiguous in _chunk_layout order."""
    n, D = xmat.shape
    KD = D // P
    xr = xmat.reshape(n, KD, P).transpose(2, 1, 0)  # [P, KD, n]
    out = np.empty((P, KD * n), BF16_NP)
    off = 0
    for c0, cw, _ in _chunk_layout(C, S):
        out[:, off : off + KD * cw] = xr[:, :, c0 : c0 + cw].reshape(P, KD * cw)
        off += KD * cw
    return out


def _pack_w13(wa, wb):
    """Two [D, H] weights -> [P, (D//P)*2H] bf16, h-tile-major with the
    w1|w3 tiles of each h paired so one DMA fetches both."""
    Dw, Hw = wa.shape
    KD = Dw // P
    KH = Hw // P
    s = np.stack(
        [wa.reshape(KD, P, KH, P), wb.reshape(KD, P, KH, P)], axis=3
    )  # [KD, P, KH, 2, P]
    return np.ascontiguousarray(
        s.transpose(1, 2, 0, 3, 4).reshape(P, KD * 2 * Hw)
    ).astype(BF16_NP)


def _pack_w2(w):
    """[H, D] -> [P, H*D//P] kh-major bf16: one contiguous run per
    partition, loadable in a single DMA."""
    Hw, Dw = w.shape
    KH = Hw // P
    return np.ascontiguousarray(
        w.reshape(KH, P, Dw).transpose(1, 0, 2).reshape(P, Hw * Dw // P)
    ).astype(BF16_NP)


def _route(xf, w_router):
    """Top-2 routing identical (up to fp rounding) to the jax reference."""
    logits = xf @ w_router.astype(np.float32)  # [T, E]
    # softmax is monotone: top-2 of probs == top-2 of logits, stable ties
    top2 = np.argsort(-logits, axis=1, kind="stable")[:, :2]  # [T, 2]
    lv = np.take_along_axis(logits, top2, axis=1)
    ev = np.exp(lv - lv[:, 0:1])
    gates = ev / ev.sum(axis=1, keepdims=True)  # [T, 2] renormalized
    return top2, gates


def kernel(x, w_router, w1, w3, w2, sw1, sw3, sw2):
    B, SEQ, D = x.shape
    T = B * SEQ
    E, _, H = w1.shape
    assert E == N_CORES
    S = T // N_CORES

    x = np.asarray(x, dtype=np.float32)
    xf = np.ascontiguousarray(x.reshape(T, D))
    top2, gates = _route(xf, np.asarray(w_router, np.float32))

    # per-expert token lists + gate values
    flat_e = top2.ravel()  # slot 2t, 2t+1 -> token t
    flat_g = gates.ravel().astype(np.float32)
    order = np.argsort(flat_e, kind="stable")
    sorted_e = flat_e[order]
    starts = np.searchsorted(sorted_e, np.arange(E + 1))
    tok_by_e = [order[starts[e] : starts[e + 1]] >> 1 for e in range(E)]
    gate_by_e = [flat_g[order[starts[e] : starts[e + 1]]] for e in range(E)]
    counts = np.diff(starts)

    # Device capacity = the mean expert load (zero padding waste); the few
    # over-capacity slots of hot experts (~1.5% here) are handled on the
    # host exactly, like a fixed-capacity MoE dispatch overflow path.
    cap = T * 2 // E
    C = max(512, min(cap, (int(counts.max()) + P - 1) // P * P))
    CP = (C + P - 1) // P * P

    nc = _get_program(D, H, C, S)

    w1 = np.asarray(w1, np.float32)
    w3 = np.asarray(w3, np.float32)
    w2 = np.asarray(w2, np.float32)
    sw13p = _pack_w13(np.asarray(sw1, np.float32), np.asarray(sw3, np.float32))
    sw2p = _pack_w2(np.asarray(sw2, np.float32))

    in_maps = []
    for e in range(E):
        n_e = min(int(counts[e]), C)
        xe_pad = np.zeros((C + S, D), np.float32)
        xe_pad[:n_e] = xf[tok_by_e[e][:n_e]]
        xe_pad[C:] = xf[e * S : (e + 1) * S]
        ge = np.zeros(CP, np.float32)
        ge[:n_e] = gate_by_e[e][:n_e]
        in_maps.append(
            {
                "xT": _pack_xT(xe_pad, C, S),
                "ge": np.ascontiguousarray(ge.reshape(CP // P, P).T),
                "w13": _pack_w13(w1[e], w3[e]),
                "w2": _pack_w2(w2[e]),
                "sw13": sw13p,
                "sw2": sw2p,
            }
        )

    global LAST_RESULTS
    LAST_RESULTS = run_bass_kernel_spmd(nc, in_maps, core_ids=list(range(N_CORES)))
    res = LAST_RESULTS.results

    out = np.empty((T, D), np.float32)
    for c in range(N_CORES):
        out[c * S : (c + 1) * S] = np.asarray(res[c]["se"], np.float32)
    for e in range(E):
        n_e = min(int(counts[e]), C)
        if n_e:
            out[tok_by_e[e][:n_e]] += np.asarray(res[e]["ye"][:n_e], np.float32)
        if int(counts[e]) > C:  # capacity-overflow slots, computed exactly
            idx = tok_by_e[e][C:]
            g = gate_by_e[e][C:]
            xo = xf[idx]
            h1 = xo @ w1[e]
            yo = (h1 / (1.0 + np.exp(-h1)) * (xo @ w3[e])) @ w2[e]
            out[idx] += yo * g[:, None]
    return out.reshape(B, SEQ, D)
